# revision 3
# baseline (speedup 1.0000x reference)
"""Trainium2 Bass kernel for BasicNonLocalBlock (N=4, C=512, H=W=64, KC=VC=256, OC=512).

Sharding: 8 cores = 4 images x 2 query-halves. Each core DMAs only ITS half of
the image (x bf16 [512,2048]), projects Q/K/V for its own 2048 pixels, then the
core pair exchanges K/V halves via per-quarter AllGather (replica groups
{2n,2n+1}); attention runs over the gathered 4096 keys (key order is
permutation-invariant under softmax). This halves the K/V projection compute
vs. recomputing the peer half locally.

All matmul operands are bf16 (bf16 rate == fp32r HIGH rate on TRN2, but bf16
halves SBUF/DMA traffic, runs DVE ops at 2x, and draws less PE power -> avoids
the HAM duty-cycle throttle that hit the f32r version's tail). PSUM stays f32.
Per-core dataflow:
  proj quarter b (512 px): K[256,512], Q[256,512] (BN + 1/sqrt(KC) folded on
    host; bias-adds on ACT), V^T[512,256] (bias on DVE) -> stage -> HBM ->
    AllGather pair -> SBUF kt/vt tiles (per-quarter-per-side, precise deps)
  attention per 512-query block, keys streamed in gathered-quarter order:
    S^T[128,512] = K_chunk^T Q_block   (PSUM, 2 matmuls)
    P^T = exp(S^T) bf16                (ACT)
    ctx[vc,512] += V^T_chunk^T P^T     (PSUM accumulation, 2 vc chunks)
    acc += P^T                         (DVE f32r accumulate, for row sums)
  sums[1,512] = ones^T @ acc; 4 PE row-transposes -> [128,4]; reciprocal;
  out^T[q,oc] = ctx^T @ WWT scaled on ACT; +bW on DVE (bf16); DMA out bf16.
Pipelining: PE warmup vs the pstate ramp, 9-deep cross-block S-matmul prefix,
quarter-level CC pipeline behind the projections. Host assembles bf16 halves
-> [4,512,64,64] f32.
"""

import sys
import types
from contextlib import ExitStack

import numpy as np

# ---------------------------------------------------------------------------
# Environment shims (axon image lacks antenv.axon_hooks; walrus rejects >2
# sync waits on the tail Drain emitted by TileContext).
# ---------------------------------------------------------------------------


def _install_ntff_hook_shim():
    try:
        import antenv
    except ImportError:
        return
    if "antenv.axon_hooks" in sys.modules:
        return
    mod = types.ModuleType("antenv.axon_hooks")
    mod._hook = None

    def set_axon_ntff_profile_hook(h):
        mod._hook = h

    def get_axon_ntff_profile_hook():
        return mod._hook

    mod.set_axon_ntff_profile_hook = set_axon_ntff_profile_hook
    mod.get_axon_ntff_profile_hook = get_axon_ntff_profile_hook
    sys.modules["antenv.axon_hooks"] = mod
    antenv.axon_hooks = mod
    try:
        if "/root/.axon_site" not in sys.path:
            sys.path.insert(0, "/root/.axon_site")
        from trn_agent_boot.trn_boot import _ntff_profile_via_ctypes

        hook = _ntff_profile_via_ctypes("/opt/axon/libaxon_pjrt.so")
        if hook is not None:
            mod._hook = hook
    except Exception:
        pass


_install_ntff_hook_shim()

import concourse.bass as bass
import concourse.bass_isa as bass_isa
import concourse.tile as tile
from concourse import mybir
from concourse.bass_utils import run_bass_kernel_spmd
from concourse.vector_clock import ScopedClock

F32 = mybir.dt.float32
F32R = mybir.dt.float32r
BF16 = mybir.dt.bfloat16
ACT = mybir.ActivationFunctionType


def _patched_drain_and_barrier(self, tick_clock, wait_clock):
    nc = self.nc
    probe = nc.sync.nop(nofuse=True, hint="drain_waits_probe")
    wait_clock.add_sem_waits(probe.ins, ScopedClock({None: tick_clock.global_clock}))
    si = probe.ins.sync_info
    waits = list(si.on_wait or []) if si is not None else []
    if si is not None:
        si.on_wait = waits[:1]
    for w in waits[1:]:
        n = nc.sync.nop(nofuse=True, hint="drain_waits_extra")
        n.ins.sync_info = mybir.SyncInfo(on_wait=[w], on_update=[])
    nc.sync.drain()
    nc.all_engine_barrier()
    assert self.sems is not None
    popped = nc._tile_sem_poison_stack.pop()
    assert popped is self._sem_poison
    nc.clear_and_free_semaphores(list(self.sems.allocated().values()))


tile.TileContext._drain_and_barrier = _patched_drain_and_barrier


def _split_excess_waits(nc):
    """Walrus CoreV3 codegen limits embedded sync waits per instruction
    (1 for self-loading Matmult's LDWEIGHTS struct, 2 elsewhere). Move the
    excess onto same-engine NOPs inserted just before."""
    n_split = 0
    for fn in nc.m.functions:
        for blk in fn.blocks:
            new_insts = []
            for inst in blk.instructions:
                max_waits = 1
                si = getattr(inst, "sync_info", None)
                if si is not None and si.on_wait and len(si.on_wait) > max_waits:
                    waits = list(si.on_wait)
                    extra = waits[:-max_waits]
                    si.on_wait = waits[-max_waits:]
                    for i in range(0, len(extra), max_waits):
                        n_split += 1
                        nop = mybir.InstNoOp(
                            name=f"{inst.name}-ws{i}",
                            engine=inst.engine,
                            ins=[], outs=[],
                            sync_info=mybir.SyncInfo(
                                on_wait=extra[i:i + max_waits], on_update=[]),
                            bass_nofuse=True,
                        )
                        new_insts.append(nop)
                new_insts.append(inst)
            blk.instructions[:] = new_insts
    return n_split

# ---------------------------------------------------------------------------
# Problem constants (hardcoded; kernel.py must be self-contained)
# ---------------------------------------------------------------------------
N_IMG, C, H, W = 4, 512, 64, 64
KC, VC, OC = 256, 256, 512
L = H * W  # 4096
QH = L // 2  # queries per core
N_CORES = 8
EPS = 1e-5

NCC = C // 128  # 4   c chunks
NKC = KC // 128  # 2  kc chunks
NVC = VC // 128  # 2  vc chunks
NKI = L // 128  # 32  key chunks
NQB = QH // 512  # 4  query blocks per core
NQT = QH // 512  # 4  pixel quarters per half

# Exchange with the pair core: keys live in "gathered" order
# [even-core pixels 0..2047, odd-core pixels 0..2047]. Key chunk ki is
# addressed as (quarter b, side, idx) and becomes available per-(b) as the
# per-quarter AllGather lands; this list is the availability order.
KI_LIST = [(b, side, i) for b in range(NQT) for side in range(2)
           for i in range(4)]

USE_CC = True  # False: recompute the peer half locally from xo (fallback)


def _bcast(ap, p=128):
    """Broadcast a 1-D DRAM AP across p partitions."""
    return bass.AP(tensor=ap.tensor, offset=ap.offset, ap=[[0, p], list(ap.ap[0])])


def _build_program(use_cc=USE_CC):
    nc = bass.Bass("TRN2", target_bir_lowering=False, debug=False,
                   num_devices=N_CORES)

    xq_ap = nc.dram_tensor("xq", [C, QH], BF16, kind="ExternalInput").ap()
    if not use_cc:
        xo_ap = nc.dram_tensor("xo", [C, QH], BF16, kind="ExternalInput").ap()
    wqT_ap = nc.dram_tensor("wqT", [C, KC], BF16, kind="ExternalInput").ap()
    wkT_ap = nc.dram_tensor("wkT", [C, KC], BF16, kind="ExternalInput").ap()
    wvT_ap = nc.dram_tensor("wvT", [C, VC], BF16, kind="ExternalInput").ap()
    wWT_ap = nc.dram_tensor("wWT", [VC, OC], BF16, kind="ExternalInput").ap()
    bq_ap = nc.dram_tensor("bq", [KC], F32, kind="ExternalInput").ap()
    bk_ap = nc.dram_tensor("bk", [KC], F32, kind="ExternalInput").ap()
    bv_ap = nc.dram_tensor("bv", [VC], F32, kind="ExternalInput").ap()
    bW_ap = nc.dram_tensor("bW", [OC], BF16, kind="ExternalInput").ap()
    out_ap = nc.dram_tensor("out_t", [QH, OC], BF16, kind="ExternalOutput").ap()

    with tile.TileContext(nc) as tc, ExitStack() as stack:
        consts = stack.enter_context(tc.tile_pool(name="consts", bufs=1))
        persist = stack.enter_context(tc.tile_pool(name="persist", bufs=1))
        mm_ps = stack.enter_context(tc.tile_pool(name="mm_ps", bufs=4,
                                                 space="PSUM"))
        ctx_psum = stack.enter_context(tc.tile_pool(name="ctx_psum", bufs=1,
                                                    space="PSUM"))
        o_psum = stack.enter_context(tc.tile_pool(name="o_psum", bufs=2,
                                                  space="PSUM"))
        acc_pool = stack.enter_context(tc.tile_pool(name="acc_sb", bufs=2))
        pt_pool = stack.enter_context(tc.tile_pool(name="pt", bufs=9))
        ctx_pool = stack.enter_context(tc.tile_pool(name="ctx_sb", bufs=2))
        o_pool = stack.enter_context(tc.tile_pool(name="o_sb", bufs=2))
        r_pool = stack.enter_context(tc.tile_pool(name="r_sb", bufs=1))
        stage_pool = stack.enter_context(tc.tile_pool(name="stage", bufs=2))
        dram_pool = stack.enter_context(tc.tile_pool(name="dramp", bufs=1,
                                                     space="DRAM"))

        # ---- weights / consts ----
        wq_s = consts.tile([128, NCC, KC], BF16, tag="wq")
        nc.sync.dma_start(wq_s[:], wqT_ap.rearrange("(a p) k -> p a k", p=128))
        wk_s = consts.tile([128, NCC, KC], BF16, tag="wk")
        nc.sync.dma_start(wk_s[:], wkT_ap.rearrange("(a p) k -> p a k", p=128))
        wv_s = consts.tile([128, NCC, VC], BF16, tag="wv")
        nc.sync.dma_start(wv_s[:], wvT_ap.rearrange("(a p) k -> p a k", p=128))
        wW_s = consts.tile([128, NVC, OC], BF16, tag="wW")
        nc.sync.dma_start(wW_s[:], wWT_ap.rearrange("(a p) k -> p a k", p=128))
        bq_s = consts.tile([128, NKC], F32, tag="bq")
        nc.sync.dma_start(bq_s[:], bq_ap.rearrange("(a p) -> p a", p=128))
        bk_s = consts.tile([128, NKC], F32, tag="bk")
        nc.sync.dma_start(bk_s[:], bk_ap.rearrange("(a p) -> p a", p=128))
        bv_s = consts.tile([128, VC], F32, tag="bv")
        nc.sync.dma_start(bv_s[:], _bcast(bv_ap))
        bW_s = consts.tile([128, OC], BF16, tag="bW")
        nc.sync.dma_start(bW_s[:], _bcast(bW_ap))
        ones_f = consts.tile([128, 1], F32, tag="onesf")
        nc.vector.memset(ones_f[:], 1.0)
        ones_b = consts.tile([128, 1], BF16, tag="onesb")
        nc.vector.tensor_copy(ones_b[:], ones_f[:])
        ones_r = consts.tile([128, 1], F32R, tag="onesr")
        nc.vector.tensor_copy(ones_r[:], ones_f[:])
        warm_exp = consts.tile([128, 1], F32, tag="wexp")
        ident1 = consts.tile([1, 1], F32, tag="id1")
        nc.vector.memset(ident1[:], 1.0)

        # ---- persistent activations (quarter/side granular for precise deps)
        # kt[b][side]: [128 kc(j), 512 keys] x NKC -> [128, NKC, 512]
        # vt[b][side]: [128 keys(idx), VC] x 4   -> [128, 4, VC]
        kt = [[persist.tile([128, NKC, 512], BF16, tag=f"kt{b}_{s}",
                            name=f"kt{b}_{s}") for s in range(2)]
              for b in range(NQT)]
        vt = [[persist.tile([128, 4, VC], BF16, tag=f"vt{b}_{s}",
                            name=f"vt{b}_{s}") for s in range(2)]
              for b in range(NQT)]
        qt = [persist.tile([128, NKC, 512], BF16, tag=f"qt{b}",
                           name=f"qt{b}") for b in range(NQT)]

        if use_cc:
            cc_in = [dram_pool.tile([128, 2048], BF16, tag=f"ccin{b}",
                                    name=f"ccin{b}") for b in range(NQT)]
            cc_out = [dram_pool.tile([256, 2048], BF16, tag=f"ccout{b}",
                                     name=f"ccout{b}") for b in range(NQT)]

        # ---- striped input DMAs -------------------------------------------
        xq_s = [[None] * NQT for _ in range(NCC)]
        xo_s = [[None] * NQT for _ in range(NCC)]

        def stripe(pool, store, src_ap, pfx, t, ci):
            xt = pool.tile([128, 512], BF16, tag=f"{pfx}{ci}_{t}",
                           name=f"{pfx}{ci}_{t}")
            nc.sync.dma_start(
                xt[:], src_ap[ci * 128:(ci + 1) * 128, t * 512:(t + 1) * 512])
            store[ci][t] = xt

        def proj_quarter(xs, b, stage_t):
            """Project one 512-pixel quarter of this core's half: K, Q (bias
            on ACT), V^T (bias on DVE). K/V go to stage_t (for the exchange)
            or directly to kt/vt side tiles when stage_t holds them."""
            for j in range(NKC):
                ps = mm_ps.tile([128, 512], F32, tag="mm", name=f"pk{j}_{b}")
                for ci in range(NCC):
                    nc.tensor.matmul(
                        ps[:],
                        wk_s[:, ci, j * 128:(j + 1) * 128],
                        xs[ci][b][:],
                        start=(ci == 0), stop=(ci == NCC - 1))
                nc.scalar.activation(stage_t[:, j * 512:(j + 1) * 512], ps[:],
                                     ACT.Identity, bias=bk_s[:, j:j + 1])
            for j in range(NKC):
                ps = mm_ps.tile([128, 512], F32, tag="mm", name=f"pq{j}_{b}")
                for ci in range(NCC):
                    nc.tensor.matmul(
                        ps[:],
                        wq_s[:, ci, j * 128:(j + 1) * 128],
                        xs[ci][b][:],
                        start=(ci == 0), stop=(ci == NCC - 1))
                nc.scalar.activation(qt[b][:, j, :], ps[:],
                                     ACT.Identity, bias=bq_s[:, j:j + 1])
            for g in range(4):
                ps = mm_ps.tile([128, VC], F32, tag="mm", name=f"pv{b}_{g}")
                for ci in range(NCC):
                    nc.tensor.matmul(
                        ps[:],
                        xs[ci][b][:, g * 128:(g + 1) * 128],
                        wv_s[:, ci, :],
                        start=(ci == 0), stop=(ci == NCC - 1))
                nc.vector.tensor_add(
                    stage_t[:, 1024 + g * VC: 1024 + (g + 1) * VC],
                    ps[:], bv_s[:])

        # ---- attention ----------------------------------------------------
        def kv_for(ki):
            b, side, idx = KI_LIST[ki]
            return (lambda j: kt[b][side][:, j, idx * 128:(idx + 1) * 128],
                    lambda j: vt[b][side][:, idx, j * 128:(j + 1) * 128])

        def attn_qblock(qb, part, state):
            if part == "prefix":
                state.setdefault("pt", {})
            elif part == "full" and "acc" not in state:
                state["acc"] = acc_pool.tile([128, 512], F32R, tag="acc",
                                             name=f"acc{qb}")
                state["ctx_ps"] = [
                    ctx_psum.tile([128, 512], F32, tag=f"ctx{j}",
                                  name=f"ctx{qb}_{j}")
                    for j in range(NVC)]
                state.setdefault("pt", {})
            acc = state.get("acc")
            ctx_ps = state.get("ctx_ps")
            pt_tiles = state["pt"]

            def emit_s(pos):
                kap, _ = kv_for(pos)
                ps = mm_ps.tile([128, 512], F32, tag="mm", name=f"s{qb}_{pos}")
                for j in range(NKC):
                    nc.tensor.matmul(
                        ps[:], kap(j), qt[qb][:, j, :],
                        start=(j == 0), stop=(j == NKC - 1))
                pt = pt_pool.tile([128, 512], BF16, tag="pt",
                                  name=f"pt{qb}_{pos}")
                nc.scalar.activation(pt[:], ps[:], ACT.Exp)
                pt_tiles[pos] = pt

            if part == "prefix":
                for pos in range(9):
                    emit_s(pos)
                return

            def emit_acc(pos):
                _, vap = kv_for(pos)
                pt = pt_tiles.pop(pos)
                if pos == 0:
                    nc.vector.tensor_copy(acc[:], pt[:])
                else:
                    nc.vector.tensor_add(acc[:], acc[:], pt[:])
                for j in range(NVC):
                    nc.tensor.matmul(
                        ctx_ps[j][:], vap(j), pt[:],
                        start=(pos == 0), stop=(pos == NKI - 1),
                        skip_group_check=True)

            if 0 not in pt_tiles:
                emit_s(0)
            for pos in range(NKI):
                if pos < NKI - 1 and pos + 1 not in pt_tiles:
                    emit_s(pos + 1)
                emit_acc(pos)
            if state.get("next") is not None:
                # pre-emit the next q-block's first S matmuls so the PE has
                # work while DVE copies ctx out of PSUM for this block
                nqb, nstate = state["next"]
                attn_qblock(nqb, "prefix", nstate)

            # softmax denominators: ones^T @ acc -> [1,512], PE row-transpose
            # to [128,4] columns, cheap reciprocal on [128,4]
            sums = mm_ps.tile([1, 512], F32, tag="mm", name=f"sbc{qb}")
            nc.tensor.matmul(sums[:], ones_r[:], acc[:],
                             start=True, stop=True, skip_group_check=True)
            srow = r_pool.tile([1, 512], F32, tag="srow", name=f"sr{qb}")
            if state.get("next") is None:   # last q-block: ACT is idle
                nc.scalar.copy(srow[:], sums[:])
            else:
                nc.vector.tensor_copy(srow[:], sums[:])
            rtp = mm_ps.tile([128, 4], F32, tag="mm", name=f"rt{qb}")
            for qs in range(4):
                nc.tensor.transpose(rtp[:, qs:qs + 1],
                                    srow[:, qs * 128:(qs + 1) * 128],
                                    ident1[:])
            rcr = r_pool.tile([128, 4], F32, tag="rcr", name=f"rcr{qb}")
            nc.vector.tensor_copy(rcr[:], rtp[:])
            rcol = r_pool.tile([128, 4], F32, tag="rcol", name=f"rc{qb}")
            nc.vector.reciprocal(rcol[:], rcr[:])

            ctx_sb = []
            for j in range(NVC):
                t = ctx_pool.tile([128, 512], BF16, tag=f"ctxs{j}",
                                  name=f"cs{qb}_{j}")
                nc.vector.tensor_copy(t[:], ctx_ps[j][:])
                ctx_sb.append(t)
            for qs in range(4):
                ops = o_psum.tile([128, OC], F32, tag="ops", name=f"o{qb}_{qs}")
                for j in range(NVC):
                    nc.tensor.matmul(
                        ops[:],
                        ctx_sb[j][:, qs * 128:(qs + 1) * 128],
                        wW_s[:, j, :],
                        start=(j == 0), stop=(j == NVC - 1))
                o_sc = o_pool.tile([128, OC], BF16, tag="osc",
                                   name=f"sc{qb}_{qs}")
                nc.scalar.activation(o_sc[:], ops[:], ACT.Copy,
                                     scale=rcol[:, qs:qs + 1])
                o_fin = o_pool.tile([128, OC], BF16, tag="ofin",
                                    name=f"of{qb}_{qs}")
                nc.vector.tensor_add(o_fin[:], o_sc[:], bW_s[:])
                nc.sync.dma_start(
                    out_ap[qb * 512 + qs * 128: qb * 512 + (qs + 1) * 128, :],
                    o_fin[:])

        # ---- program order ------------------------------------------------
        with tc.tile_pool(name="xqpool", bufs=1) as xqp, \
                tc.tile_pool(name="xopool", bufs=1) as xop:
            # PE warm-up on the (tiny, early) weight tiles: start the pstate
            # ramp before the projections. Emitted before the stripe DMAs so
            # its queue-sem waits don't cover them.
            for wi in range(12):
                wps = mm_ps.tile([1, KC], F32, tag="mm", name=f"warm{wi}")
                nc.tensor.matmul(wps[:], ones_b[:], wq_s[:, 0, :],
                                 start=True, stop=True, skip_group_check=True)
            for t in range(NQT):
                for ci in range(NCC):
                    stripe(xqp, xq_s, xq_ap, "xq", t, ci)
            if not use_cc:
                for t in range(NQT):
                    for ci in range(NCC):
                        stripe(xop, xo_s, xo_ap, "xo", t, ci)

            if use_cc:
                for b in range(NQT):
                    st = stage_pool.tile([128, 2048], BF16, tag="stage",
                                         name=f"stage{b}")
                    proj_quarter(xq_s, b, st)
                    nc.sync.dma_start(cc_in[b][:], st[:])
                    nc.gpsimd.collective_compute(
                        "AllGather",
                        mybir.AluOpType.bypass,
                        replica_groups=[[0, 1], [2, 3], [4, 5], [6, 7]],
                        ins=[cc_in[b][:].opt()],
                        outs=[cc_out[b][:].opt()],
                    )
                    for side in range(2):
                        r0 = side * 128
                        nc.gpsimd.dma_start(
                            kt[b][side][:], cc_out[b][r0:r0 + 128, 0:1024])
                        nc.gpsimd.dma_start(
                            vt[b][side][:], cc_out[b][r0:r0 + 128, 1024:2048])
            else:
                # fallback: recompute the peer half locally (as the f32r
                # baseline did), writing sides directly
                for b in range(NQT):
                    own = persist.tile([128, 2048], BF16, tag=f"own{b}",
                                       name=f"own{b}")
                    proj_quarter(xq_s, b, own)
                    nc.vector.tensor_copy(kt[b][0][:],
                                          own[:, 0:1024].rearrange(
                                              "p (j f) -> p j f", j=NKC))
                    nc.vector.tensor_copy(vt[b][0][:],
                                          own[:, 1024:2048].rearrange(
                                              "p (g f) -> p g f", g=4))
                for b in range(NQT):
                    oth = persist.tile([128, 2048], BF16, tag=f"oth{b}",
                                       name=f"oth{b}")
                    proj_quarter(xo_s, b, oth)
                    nc.vector.tensor_copy(kt[b][1][:],
                                          oth[:, 0:1024].rearrange(
                                              "p (j f) -> p j f", j=NKC))
                    nc.vector.tensor_copy(vt[b][1][:],
                                          oth[:, 1024:2048].rearrange(
                                              "p (g f) -> p g f", g=4))

            # preload the Exp LUT (attention's first exp skips table load);
            # after ALL proj bias-adds (Identity) to avoid ACT table thrash
            nc.scalar.activation(warm_exp[:], ones_f[:], ACT.Exp)

            states = [{} for _ in range(NQB)]
            for qb in range(NQB - 1):
                states[qb]["next"] = (qb + 1, states[qb + 1])
            states[NQB - 1]["next"] = None
            attn_qblock(0, "full", states[0])
        for qb in range(1, NQB):
            attn_qblock(qb, "full", states[qb])

    _split_excess_waits(nc)
    return nc


_NC_CACHE = {}


def _get_nc():
    if "nc" not in _NC_CACHE:
        _NC_CACHE["nc"] = _build_program()
    return _NC_CACHE["nc"]


def _prep_in_maps(x, wq, bq, gq, betaq, mq, vq, wk, bk, gk, betak, mk, vk,
                  wv, bv, wW, bW):
    bf = mybir.dt.np(BF16)
    x = np.asarray(x, np.float32)
    invq = np.asarray(gq, np.float32) / np.sqrt(np.asarray(vq, np.float32) + EPS)
    invk = np.asarray(gk, np.float32) / np.sqrt(np.asarray(vk, np.float32) + EPS)
    scale = 1.0 / np.sqrt(np.float32(KC))
    wq_f = (np.asarray(wq, np.float32) * invq[:, None]) * scale
    bq_f = (np.asarray(bq, np.float32) * invq + np.asarray(betaq, np.float32)
            - np.asarray(mq, np.float32) * invq) * scale
    wk_f = np.asarray(wk, np.float32) * invk[:, None]
    bk_f = (np.asarray(bk, np.float32) * invk + np.asarray(betak, np.float32)
            - np.asarray(mk, np.float32) * invk)

    shared = {
        "wqT": np.ascontiguousarray(wq_f.T).astype(bf),
        "wkT": np.ascontiguousarray(wk_f.T).astype(bf),
        "wvT": np.ascontiguousarray(np.asarray(wv, np.float32).T).astype(bf),
        "wWT": np.ascontiguousarray(np.asarray(wW, np.float32).T).astype(bf),
        "bq": np.ascontiguousarray(bq_f, np.float32),
        "bk": np.ascontiguousarray(bk_f, np.float32),
        "bv": np.ascontiguousarray(np.asarray(bv, np.float32)),
        "bW": np.asarray(bW, np.float32).astype(bf),
    }
    in_maps = []
    for c in range(N_CORES):
        n, half = c // 2, c % 2
        x_img = x[n].reshape(C, L)
        xq = np.ascontiguousarray(
            x_img[:, half * QH:(half + 1) * QH]).astype(bf)
        m = {"xq": xq, **shared}
        if not USE_CC:
            m["xo"] = np.ascontiguousarray(
                x_img[:, (1 - half) * QH:(2 - half) * QH]).astype(bf)
        in_maps.append(m)
    return in_maps


def _assemble(results):
    full = np.empty((N_IMG, OC, L), np.float32)
    for n in range(N_IMG):
        halves = [np.asarray(results[2 * n]["out_t"], np.float32),
                  np.asarray(results[2 * n + 1]["out_t"], np.float32)]
        img = np.concatenate(halves, axis=0)  # [L, OC]
        full[n] = img.T
    return full.reshape(N_IMG, OC, H, W)


def run_bass(trace=False, **inputs):
    nc = _get_nc()
    in_maps = _prep_in_maps(**inputs)
    res = run_bass_kernel_spmd(nc, in_maps, core_ids=list(range(N_CORES)),
                               trace=trace)
    return _assemble(res.results), res


def kernel(**inputs):
    out, _ = run_bass(trace=False, **inputs)
    return out
